# revision 1
# baseline (speedup 1.0000x reference)
"""Trainium2 Bass kernel for a GPT-2 style transformer block
(S=3072, E=1024, 16 heads, MLP 4x), distributed over 8 NeuronCores.

Sharding:
  - LN1 runs sequence-parallel (each core normalizes+transposes its 384-row
    chunk), then an AllGather gives every core the full [E, S] normalized,
    transposed activations (bf16).
  - Attention is tensor-parallel over heads (2 heads/core).
  - One AllToAll reshards attention output to sequence-parallel chunks;
    proj + residual + LN2 + the full MLP run per-chunk with no further
    collectives. The host concatenates the 8 output chunks.

On-device layout is "transposed activations" [feature-partition, seq-free]:
every matmul contracts over the partition dim, and the causal softmax needs
no row-max pass (scores are O(1); masked lanes of the diagonal band are
zeroed post-exp by a gpsimd affine_select; fully-masked blocks are skipped).
The softmax denominator comes free from a ones-augmented V column. Weights
are pre-tiled on the host so every DMA is contiguous. Matmuls run as
float32r (scores/AV/proj) and bfloat16 (qkv, MLP) with fp32 PSUM
accumulation.
"""

import numpy as np

E, H, I = 1024, 16, 4096
W = 8
MASK = -10000.0

_CACHE = {}


def _build(SS: int, dt_mm_name: str, mock_cc: bool = False):
    """Build the SPMD Bass program for sequence length SS.
    dt_mm_name: 'float32r' (fast) or 'float32' (exact) for the fp32-operand
    matmuls (scores, AV, proj)."""
    import concourse.mybir as mybir
    import concourse.tile as tile
    from concourse import bacc
    from concourse.masks import make_identity

    f32 = mybir.dt.float32
    bf16 = mybir.dt.bfloat16
    dt_mm = getattr(mybir.dt, dt_mm_name)
    AF = mybir.ActivationFunctionType
    ALU = mybir.AluOpType
    X = mybir.AxisListType.X

    CH = SS // W          # seq rows per core
    NB = SS // 128        # 128-blocks along full sequence
    B = CH // 128         # 128-blocks per chunk

    nc = bacc.Bacc(None)

    hid = nc.dram_tensor("hidden", [CH, E], f32, kind="ExternalInput")
    qkv_w = nc.dram_tensor("qkv_w", [128, 3 * 8 * 128], bf16, kind="ExternalInput")
    qkv_b = nc.dram_tensor("qkv_b", [128, 3], f32, kind="ExternalInput")
    proj_w = nc.dram_tensor("proj_w", [8 * 128, 8 * 128], dt_mm, kind="ExternalInput")
    proj_b = nc.dram_tensor("proj_b", [128, 8], f32, kind="ExternalInput")
    ln1_w = nc.dram_tensor("ln1_w", [128, 8], f32, kind="ExternalInput")
    ln1_b = nc.dram_tensor("ln1_b", [128, 8], f32, kind="ExternalInput")
    ln2_w = nc.dram_tensor("ln2_w", [128, 8], f32, kind="ExternalInput")
    ln2_b = nc.dram_tensor("ln2_b", [128, 8], f32, kind="ExternalInput")
    w1 = nc.dram_tensor("w1", [32 * 128, 8 * 128], bf16, kind="ExternalInput")
    b1 = nc.dram_tensor("b1", [128, 32], f32, kind="ExternalInput")
    w2 = nc.dram_tensor("w2", [8 * 128, 32 * 128], bf16, kind="ExternalInput")
    b2 = nc.dram_tensor("b2", [128, 8], f32, kind="ExternalInput")
    mask_b = nc.dram_tensor("mask_bias", [128, NB], f32, kind="ExternalInput")
    out = nc.dram_tensor("out", [CH, E], f32, kind="ExternalOutput")

    rg = [list(range(W))]

    with tile.TileContext(nc) as tc:
        with (
            tc.tile_pool(name="dram", bufs=1, space="DRAM") as dram,
            tc.tile_pool(name="const", bufs=1) as const,
            tc.tile_pool(name="persist", bufs=1) as persist,
            tc.tile_pool(name="work", bufs=2) as work,
            tc.tile_pool(name="xgp", bufs=8) as xgp,
            tc.tile_pool(name="exp", bufs=4) as exp_pool,
            tc.tile_pool(name="psum", bufs=2, space="PSUM") as psum,
            tc.tile_pool(name="psacc", bufs=2, space="PSUM") as psacc,
        ):
            # ----- constants -----
            ident = const.tile([128, 128], f32, tag="ident", name="ident")
            make_identity(nc, ident[:])
            ident_bf = const.tile([128, 128], bf16, tag="identbf", name="identbf")
            nc.vector.tensor_copy(ident_bf[:], ident[:])
            eps_sb = const.tile([128, 1], f32, tag="eps", name="eps")
            nc.vector.memset(eps_sb[:], 1e-5)

            def load2d(dram_t, shape, name):
                t = const.tile(shape, f32, tag=name, name=name)
                nc.sync.dma_start(out=t[:], in_=dram_t[:, :])
                return t

            ln1_w_sb = load2d(ln1_w, [128, 8], "ln1w")
            ln1_b_sb = load2d(ln1_b, [128, 8], "ln1b")
            ln2_w_sb = load2d(ln2_w, [128, 8], "ln2w")
            ln2_b_sb = load2d(ln2_b, [128, 8], "ln2b")
            qkv_b_sb = load2d(qkv_b, [128, 3], "qkvb")
            proj_b_sb = load2d(proj_b, [128, 8], "projb")
            b1_sb = load2d(b1, [128, 32], "b1")
            b2_sb = load2d(b2, [128, 8], "b2")
            mb_sb = load2d(mask_b, [128, NB], "maskbias")

            def dma(out_, in_):
                return nc.sync.dma_start(out=out_, in_=in_)

            # ----- LN (row layout) + transpose; w/b applied post-transpose -----
            def layer_norm_T(x_tiles, w_sb, b_sb, out_dt):
                xT = [persist.tile([128, CH], out_dt, tag=f"lnT{k}", name=f"lnT{k}")
                      for k in range(8)]
                for t in range(len(x_tiles)):
                    x = x_tiles[t]
                    stat = work.tile([128, 8], f32, tag="lnstat", name="lnstat")
                    scr = work.tile([128, E], f32, tag="lnscr", name="lnscr")
                    nc.vector.reduce_sum(out=stat[:, 0:1], in_=x[:], axis=X)
                    nc.vector.tensor_scalar_mul(stat[:, 1:2], stat[:, 0:1], 1.0 / E)
                    nc.scalar.activation(scr[:], x[:], AF.Square,
                                         accum_out=stat[:, 2:3])
                    nc.vector.tensor_scalar_mul(stat[:, 2:3], stat[:, 2:3], 1.0 / E)
                    nc.vector.tensor_tensor(out=stat[:, 3:4], in0=stat[:, 1:2],
                                            in1=stat[:, 1:2], op=ALU.mult)
                    nc.vector.tensor_tensor(out=stat[:, 3:4], in0=stat[:, 2:3],
                                            in1=stat[:, 3:4], op=ALU.subtract)
                    nc.scalar.activation(stat[:, 4:5], stat[:, 3:4], AF.Sqrt,
                                         bias=eps_sb[:], scale=1.0)
                    nc.vector.reciprocal(out=stat[:, 4:5], in_=stat[:, 4:5])
                    nc.vector.tensor_tensor(out=stat[:, 5:6], in0=stat[:, 1:2],
                                            in1=stat[:, 4:5], op=ALU.mult)
                    nc.vector.tensor_scalar_mul(stat[:, 5:6], stat[:, 5:6], -1.0)
                    xn = work.tile([128, E], f32, tag="lnscr", name="lnxn")
                    nc.vector.tensor_scalar(out=xn[:], in0=x[:],
                                            scalar1=stat[:, 4:5],
                                            scalar2=stat[:, 5:6],
                                            op0=ALU.mult, op1=ALU.add)
                    for m in range(8):
                        tp = psum.tile([128, 128], f32, tag="tp", name="tp")
                        nc.tensor.transpose(tp[:], xn[:, m * 128:(m + 1) * 128],
                                            ident[:])
                        nc.vector.tensor_scalar(
                            out=xT[m][:, t * 128:(t + 1) * 128], in0=tp[:],
                            scalar1=w_sb[:, m:m + 1], scalar2=b_sb[:, m:m + 1],
                            op0=ALU.mult, op1=ALU.add)
                return xT

            # ----- stage 1: LN1 on own chunk -----
            x_rows = []
            for t in range(B):
                xt = persist.tile([128, E], f32, tag=f"xrow{t}", name=f"xrow{t}")
                dma(xt[:], hid[t * 128:(t + 1) * 128, :])
                x_rows.append(xt)
            xnT = layer_norm_T(x_rows, ln1_w_sb, ln1_b_sb, bf16)

            # ----- stage 2: AllGather normalized-transposed chunks (bf16) -----
            ag_in = dram.tile([E, CH], bf16)
            ag_out = dram.tile([W * E, CH], bf16, addr_space="Shared")
            for m in range(8):
                dma(ag_in[m * 128:(m + 1) * 128, :], xnT[m][:])
            if mock_cc:
                nc.sync.dma_start(out=ag_out[0:E, :], in_=ag_in[:, :])
            else:
                nc.gpsimd.collective_compute(
                    "AllGather", ALU.bypass, replica_groups=rg,
                    ins=[ag_in.opt()], outs=[ag_out.opt()])

            # ----- stage 3: qkv for this core's 2 heads (bf16 matmuls) -----
            wqkv = persist.tile([128, 3 * 8 * 128], bf16, tag="wqkv", name="wqkv")
            dma(wqkv[:], qkv_w[:, :])

            qT = persist.tile([128, SS], dt_mm, tag="qT", name="qT")
            kT = persist.tile([128, SS], dt_mm, tag="kT", name="kT")
            vT = persist.tile([128, SS], f32, tag="vT", name="vT")
            qkvT = [qT, kT, vT]
            for j in range(W):
                xg = [xgp.tile([128, CH], bf16, tag="xg", name="xg")
                      for _ in range(8)]
                for k in range(8):
                    dma(xg[k][:], ag_out[j * E + k * 128:j * E + (k + 1) * 128, :])
                for c in range(3):
                    ps = psacc.tile([128, CH], f32, tag="mmacc", name="mmacc")
                    for k in range(8):
                        nc.tensor.matmul(
                            ps[:],
                            lhsT=wqkv[:, (c * 8 + k) * 128:(c * 8 + k + 1) * 128],
                            rhs=xg[k][:],
                            start=(k == 0), stop=(k == 7))
                    nc.scalar.activation(
                        qkvT[c][:, j * CH:(j + 1) * CH], ps[:], AF.Identity,
                        bias=qkv_b_sb[:, c:c + 1], scale=1.0)

            # ----- stage 4: V transposed + ones-augmented column -----
            v_aug = [persist.tile([128, NB * 65], dt_mm, tag=f"vaug{h}",
                                  name=f"vaug{h}") for h in range(2)]
            ones128 = nc.const_aps.tensor(1.0, (128, 1))
            for h in range(2):
                for tb in range(NB):
                    nc.scalar.copy(
                        v_aug[h][:, tb * 65 + 64:tb * 65 + 65], ones128)
            for tb in range(NB):
                # one transpose covers both heads: out rows=t, cols=(h0 e, h1 e)
                tp = psum.tile([128, 128], f32, tag="tp", name="vtp")
                nc.tensor.transpose(tp[:], vT[:, tb * 128:(tb + 1) * 128], ident[:])
                for h in range(2):
                    nc.vector.tensor_copy(v_aug[h][:, tb * 65:tb * 65 + 64],
                                          tp[:, 64 * h:64 * h + 64])

            # ----- stage 5: attention (h-outer) + per-head AllToAll so the
            # head-0 collective overlaps head-1 compute -----
            a2a_in = [dram.tile([W * 64, CH], dt_mm, name=f"a2a_in{h}")
                      for h in range(2)]
            a2a_out = [dram.tile([W * 64, CH], dt_mm, name=f"a2a_out{h}")
                       for h in range(2)]
            for h in range(2):
                e0 = 64 * h
                for j in range(W):
                    n_t = B * (j + 1)
                    av = psacc.tile([65, CH], f32, tag="avacc", name="avacc")
                    for tb in range(n_t):
                        sc = psum.tile([128, CH], f32, tag="sc", name="sc")
                        nc.tensor.matmul(
                            sc[:],
                            lhsT=kT[e0:e0 + 64, tb * 128:(tb + 1) * 128],
                            rhs=qT[e0:e0 + 64, j * CH:(j + 1) * CH],
                            start=True, stop=True)
                        ex = exp_pool.tile([128, CH], dt_mm, tag="ex", name="ex")
                        nc.scalar.activation(ex[:], sc[:], AF.Exp,
                                             bias=mb_sb[:, tb:tb + 1], scale=1.0)
                        p = tb - B * j
                        if p >= 0:
                            # zero the future (t > s) lanes of the diagonal band
                            nc.gpsimd.affine_select(
                                out=ex[:], in_=ex[:],
                                compare_op=ALU.is_ge,   # keep where s-t-128p >= 0
                                fill=0.0,
                                base=-128 * p,
                                channel_multiplier=-1,
                                pattern=[[1, CH]],
                            )
                        nc.tensor.matmul(
                            av[:],
                            lhsT=v_aug[h][:, tb * 65:(tb + 1) * 65],
                            rhs=ex[:],
                            start=(tb == 0), stop=(tb == n_t - 1))
                    recip = work.tile([1, CH], f32, tag="recip", name="recip")
                    nc.vector.reciprocal(out=recip[:], in_=av[64:65, :])
                    bc = work.tile([64, CH], f32, tag="bc", name="bc")
                    nc.gpsimd.partition_broadcast(bc[:], recip[:])
                    avs = work.tile([64, CH], dt_mm, tag="avsc", name="avsc")
                    nc.vector.tensor_tensor(out=avs[:], in0=av[0:64, :],
                                            in1=bc[:], op=ALU.mult)
                    dma(a2a_in[h][j * 64:(j + 1) * 64, :], avs[:])
                # ----- stage 6: AllToAll for this head's slices -----
                if mock_cc:
                    nc.sync.dma_start(out=a2a_out[h][:, :], in_=a2a_in[h][:, :])
                else:
                    nc.gpsimd.collective_compute(
                        "AllToAll", ALU.bypass, replica_groups=rg,
                        ins=[a2a_in[h].opt()], outs=[a2a_out[h].opt()])

            # ----- stage 7: proj + residual -----
            aT = [persist.tile([128, CH], dt_mm, tag=f"aT{k}", name=f"aT{k}")
                  for k in range(8)]
            for k in range(8):
                dma(aT[k][0:64, :], a2a_out[0][k * 64:(k + 1) * 64, :])
                dma(aT[k][64:128, :], a2a_out[1][k * 64:(k + 1) * 64, :])
            res1 = [persist.tile([128, E], f32, tag=f"res1_{t}", name=f"res1_{t}")
                    for t in range(B)]
            for m in range(8):
                wpm = work.tile([128, 8 * 128], dt_mm, tag="wpm", name="wpm",
                                bufs=3)
                dma(wpm[:], proj_w[m * 128:(m + 1) * 128, :])
                ps = psacc.tile([128, CH], f32, tag="mmacc", name="mmacc")
                for k in range(8):
                    nc.tensor.matmul(
                        ps[:], lhsT=wpm[:, k * 128:(k + 1) * 128],
                        rhs=aT[k][:], start=(k == 0), stop=(k == 7))
                pTm = work.tile([128, CH], f32, tag="pTm", name="pTm")
                nc.vector.tensor_scalar(
                    out=pTm[:], in0=ps[:], scalar1=proj_b_sb[:, m:m + 1],
                    scalar2=None, op0=ALU.add)
                for t in range(B):
                    tp = psum.tile([128, 128], f32, tag="tp", name="tp")
                    nc.tensor.transpose(tp[:], pTm[:, t * 128:(t + 1) * 128],
                                        ident[:])
                    nc.vector.tensor_tensor(
                        out=res1[t][:, m * 128:(m + 1) * 128],
                        in0=tp[:], in1=x_rows[t][:, m * 128:(m + 1) * 128],
                        op=ALU.add)

            # ----- stage 8: LN2 (output tiles bf16 for the MLP) -----
            l2T = layer_norm_T(res1, ln2_w_sb, ln2_b_sb, bf16)

            # ----- stage 9: MLP (full, on this core's seq chunk; bf16) -----
            h1T = [persist.tile([128, CH], bf16, tag=f"h1T{m}", name=f"h1T{m}")
                   for m in range(32)]
            for m in range(32):
                w1m = work.tile([128, 8 * 128], bf16, tag="w1m", name="w1m",
                                bufs=4)
                dma(w1m[:], w1[m * 128:(m + 1) * 128, :])
                ps = psacc.tile([128, CH], f32, tag="mmacc", name="mmacc")
                for k in range(8):
                    nc.tensor.matmul(
                        ps[:], lhsT=w1m[:, k * 128:(k + 1) * 128],
                        rhs=l2T[k][:], start=(k == 0), stop=(k == 7))
                nc.vector.tensor_scalar(
                    out=h1T[m][:], in0=ps[:], scalar1=b1_sb[:, m:m + 1],
                    scalar2=0.0, op0=ALU.add, op1=ALU.max)

            oT = [persist.tile([128, CH], bf16, tag=f"oT{m}", name=f"oT{m}")
                  for m in range(8)]
            for m in range(8):
                ps = psacc.tile([128, CH], f32, tag="mmacc", name="mmacc")
                for half in range(2):
                    w2m = work.tile([128, 16 * 128], bf16, tag="w2m", name="w2m", bufs=3)
                    dma(w2m[:], w2[m * 128:(m + 1) * 128,
                                   half * 16 * 128:(half + 1) * 16 * 128])
                    for kk in range(16):
                        k = half * 16 + kk
                        nc.tensor.matmul(
                            ps[:], lhsT=w2m[:, kk * 128:(kk + 1) * 128],
                            rhs=h1T[k][:], start=(k == 0), stop=(k == 31))
                nc.vector.tensor_scalar(
                    out=oT[m][:], in0=ps[:], scalar1=b2_sb[:, m:m + 1],
                    scalar2=None, op0=ALU.add)

            # ----- stage 10: transpose back + final residual + out -----
            for t in range(B):
                orow = work.tile([128, E], f32, tag="orow", name="orow")
                for m in range(8):
                    tp = psum.tile([128, 128], bf16, tag="tp", name="tp")
                    nc.tensor.transpose(tp[:], oT[m][:, t * 128:(t + 1) * 128],
                                        ident_bf[:])
                    nc.vector.tensor_tensor(
                        out=orow[:, m * 128:(m + 1) * 128],
                        in0=tp[:], in1=res1[t][:, m * 128:(m + 1) * 128],
                        op=ALU.add)
                dma(out[t * 128:(t + 1) * 128, :], orow[:])

    return nc


def _prepare_in_maps(inputs, SS: int):
    """Host-side prep: slice per core, prescale q by 1/8, pre-tile all weight
    matrices so every device DMA is contiguous; bf16-cast qkv/mlp weights."""
    import ml_dtypes

    bf16 = ml_dtypes.bfloat16
    CH = SS // W
    NB = SS // 128
    hid = np.ascontiguousarray(
        np.asarray(inputs["hidden_states"], np.float32)[0, :SS])
    attn_w = np.asarray(inputs["attn_w"], np.float32).copy()
    attn_b = np.asarray(inputs["attn_b"], np.float32).copy()
    attn_w[:, :E] *= 0.125
    attn_b[:E] *= 0.125
    mask = np.asarray(inputs["mask"])[0, 0, 0, :SS]
    mask_bias = np.where(mask, 0.0, MASK).astype(np.float32)

    def vec2d(v, n):
        return np.ascontiguousarray(
            np.asarray(v, np.float32)[:n].reshape(n // 128, 128).T)

    proj_w = np.asarray(inputs["proj_w"], np.float32)
    w1 = np.asarray(inputs["mlp_w1"], np.float32)
    w2 = np.asarray(inputs["mlp_w2"], np.float32)

    # X[k*128+p, m*128+f] -> [(m p), (k f)]
    def tile_mk(x, km, mm_):
        return np.ascontiguousarray(
            x.reshape(km, 128, mm_, 128).transpose(2, 1, 0, 3)
            .reshape(mm_ * 128, km * 128))

    common = {
        "proj_w": tile_mk(proj_w, 8, 8),
        "proj_b": vec2d(inputs["proj_b"], E),
        "ln1_w": vec2d(inputs["ln1_w"], E),
        "ln1_b": vec2d(inputs["ln1_b"], E),
        "ln2_w": vec2d(inputs["ln2_w"], E),
        "ln2_b": vec2d(inputs["ln2_b"], E),
        "w1": tile_mk(w1, 8, 32).astype(bf16),
        "b1": vec2d(inputs["mlp_b1"], I),
        "w2": tile_mk(w2, 32, 8).astype(bf16),
        "b2": vec2d(inputs["mlp_b2"], E),
        "mask_bias": np.ascontiguousarray(mask_bias.reshape(NB, 128).T),
    }
    in_maps = []
    for i in range(W):
        wq = np.empty((128, 3, 8, 128), np.float32)
        bq = np.empty((128, 3), np.float32)
        for c in range(3):
            cols = slice(c * E + 128 * i, c * E + 128 * i + 128)
            wq[:, c] = attn_w[:, cols].reshape(8, 128, 128).transpose(1, 0, 2)
            bq[:, c] = attn_b[cols]
        in_maps.append({
            "hidden": np.ascontiguousarray(hid[i * CH:(i + 1) * CH]),
            "qkv_w": np.ascontiguousarray(wq.reshape(128, -1)).astype(bf16),
            "qkv_b": np.ascontiguousarray(bq),
            **common,
        })
    return in_maps


def _run(inputs, SS, dt_mm, **kw):
    from concourse.bass_utils import run_bass_kernel_spmd

    key = (SS, dt_mm)
    if key not in _CACHE:
        nc = _build(SS, dt_mm)
        nc.finalize()
        _CACHE[key] = nc
    nc = _CACHE[key]
    in_maps = _prepare_in_maps(inputs, SS)
    res = run_bass_kernel_spmd(nc, in_maps, core_ids=list(range(W)), **kw)
    full = np.concatenate([r["out"] for r in res.results], axis=0)
    return full[None].astype(np.float32), res


def kernel(**inputs) -> np.ndarray:
    out, _ = _run(inputs, 3072, "float32r")
    return out



# revision 5
# speedup vs baseline: 1.7251x; 1.7251x over previous
"""Trainium2 Bass kernel for a GPT-2 style transformer block
(S=3072, E=1024, 16 heads, MLP 4x), distributed over 8 NeuronCores.

Sharding (unchanged from v1):
  - LN1 sequence-parallel, AllGather of normalized+transposed activations
    (bf16), attention tensor-parallel over heads (2/core), per-head AllToAll
    back to sequence-parallel, then proj+LN2+MLP per seq chunk.

v2 performance restructure:
  - LN affine weights folded into the following matmul weights on the host
    (exact), so LN emits plain normalized values and the per-block
    scale/bias DVE ops disappear.
  - bf16 everywhere on device (activations, scores, attention probs, all
    collective payloads); fp32 only in PSUM accumulation, LN stats and
    residuals.
  - Attention processes 512-wide query chunks; scores for two consecutive
    key blocks land in one 2-bank PSUM tile and are exponentiated by a
    single ACT instruction (the 352-cycle ACT fixed cost is the attention
    bottleneck).  Causal masking of diagonal blocks stays post-exp on
    GPSIMD (affine_select), one call per block pair.
  - The external mask enters as the per-partition ACT bias only when the
    mask actually masks something (separate cached program variant);
    biases similarly get matmul-group ones-outer-product adds only when
    nonzero.
  - Head-outer attention loop so head 0's AllToAll overlaps head 1's
    compute.
  - Batched PSUM evacuations (4 transposed 128-blocks per copy), engine
    alternation between ACT and DVE for all PSUM->SBUF moves.
  - MLP weights streamed with deep buffering; per-core HBM traffic is
    ~20 MB against ~85 us of compute, so the stream hides completely.
"""

import numpy as np

E, H, I = 1024, 16, 4096
W = 8
MASK = -10000.0
QC = 512  # query-chunk width (one PSUM bank of fp32)

_CACHE = {}


def _build(SS: int, dt_name: str, masked: bool = False,
           biases=(False, False, False, False), mock_cc: bool = False):
    """Build the SPMD Bass program.
    dt_name: 'bf16' (fast, HW) or 'float32' (attention in fp32, for sim).
    masked: external mask has False entries -> per-block exp bias path.
    biases: (qkv, proj, mlp1, mlp2) nonzero-bias flags."""
    import concourse.mybir as mybir
    import concourse.tile as tile
    from concourse import bacc
    from concourse.masks import make_identity

    f32 = mybir.dt.float32
    bf16 = mybir.dt.bfloat16
    dt_act = bf16 if dt_name == "bf16" else f32
    AF = mybir.ActivationFunctionType
    ALU = mybir.AluOpType
    X = mybir.AxisListType.X

    CH = SS // W          # seq rows per core
    NB = SS // 128        # 128-blocks along full sequence
    B = CH // 128         # 128-blocks per chunk
    NQ = SS // QC         # 512-wide query chunks
    use_qkv_b, use_proj_b, use_mlp1_b, use_mlp2_b = biases

    nc = bacc.Bacc(None)

    hid = nc.dram_tensor("hidden", [CH, E], f32, kind="ExternalInput")
    qkv_w = nc.dram_tensor("qkv_w", [128, 3 * 8 * 128], bf16, kind="ExternalInput")
    qkv_b = nc.dram_tensor("qkv_b", [1, 3 * 128], f32, kind="ExternalInput")
    proj_w = nc.dram_tensor("proj_w", [128, 8 * 8 * 128], bf16, kind="ExternalInput")
    proj_b = nc.dram_tensor("proj_b", [1, 8 * 128], f32, kind="ExternalInput")
    w1 = nc.dram_tensor("w1", [32 * 128, 8 * 128], bf16, kind="ExternalInput")
    b1 = nc.dram_tensor("b1", [1, 32 * 128], f32, kind="ExternalInput")
    w2 = nc.dram_tensor("w2", [8 * 128, 32 * 128], bf16, kind="ExternalInput")
    b2 = nc.dram_tensor("b2", [1, 8 * 128], f32, kind="ExternalInput")
    mask_b = nc.dram_tensor("mask_bias", [128, NB], f32, kind="ExternalInput")
    out = nc.dram_tensor("out", [CH, E], f32, kind="ExternalOutput")

    rg = [list(range(W))]

    with tile.TileContext(nc) as tc:
        with (
            tc.tile_pool(name="dram", bufs=1, space="DRAM") as dram,
            tc.tile_pool(name="const", bufs=1) as const,
            tc.tile_pool(name="persist", bufs=1) as persist,
            tc.tile_pool(name="work", bufs=2) as work,
            tc.tile_pool(name="xcc", bufs=2) as xccp,
            tc.tile_pool(name="exp", bufs=3) as exp_pool,
            tc.tile_pool(name="wstream", bufs=6) as wstream,
            tc.tile_pool(name="w2stream", bufs=3) as w2stream,
            tc.tile_pool(name="tp", bufs=2, space="PSUM") as tp_pool,
            tc.tile_pool(name="sc", bufs=2, space="PSUM") as sc_pool,
            tc.tile_pool(name="acc", bufs=2, space="PSUM") as acc_pool,
        ):
            # ----- constants -----
            ident = const.tile([128, 128], f32, tag="ident", name="ident")
            make_identity(nc, ident[:])
            ident_bf = const.tile([128, 128], bf16, tag="identbf", name="identbf")
            nc.vector.tensor_copy(ident_bf[:], ident[:])
            ident_a = ident_bf if dt_act != f32 else ident
            eps_sb = const.tile([128, 1], f32, tag="eps", name="eps")
            nc.vector.memset(eps_sb[:], 1e-5)
            ones_row = const.tile([1, QC], bf16, tag="ones_row", name="ones_row")
            nc.vector.memset(ones_row[:], 1.0)

            mb_sb = None
            if masked:
                mb_sb = const.tile([128, NB], f32, tag="maskbias", name="maskbias")
                nc.sync.dma_start(out=mb_sb[:], in_=mask_b[:, :])

            def loadb(dram_t, n, name):
                t = const.tile([1, n], f32, tag=name, name=name)
                nc.sync.dma_start(out=t[:], in_=dram_t[:, :])
                return t

            qkv_b_sb = loadb(qkv_b, 3 * 128, "qkvb") if use_qkv_b else None
            proj_b_sb = loadb(proj_b, 8 * 128, "projb") if use_proj_b else None
            b1_sb = loadb(b1, 32 * 128, "b1") if use_mlp1_b else None
            b2_sb = loadb(b2, 8 * 128, "b2") if use_mlp2_b else None

            def dma(out_, in_):
                return nc.sync.dma_start(out=out_, in_=in_)

            # engine-alternating PSUM->SBUF evacuation
            ev_state = [0]

            def evac(out_, in_):
                ev_state[0] ^= 1
                if ev_state[0]:
                    nc.scalar.copy(out_, in_)
                else:
                    nc.vector.tensor_copy(out=out_, in_=in_)

            # ----- layer norm (plain: affine folded into next weights) -----
            def layer_norm_T(x, xnT, pfx):
                """x: [128, B*1024] rows tile; writes xnT [128, 8*CH] bf16
                (feature-block-major, seq-minor)."""
                ssum = work.tile([128, B], f32, tag="lnsum", name=f"{pfx}sum")
                ssq = work.tile([128, B], f32, tag="lnsq", name=f"{pfx}sq")
                for t in range(B):
                    xs = x[:, t * E:(t + 1) * E]
                    nc.vector.reduce_sum(out=ssum[:, t:t + 1], in_=xs, axis=X)
                    scr = work.tile([128, E], f32, tag="lnscr", name="lnscr")
                    nc.scalar.activation(scr[:], xs, AF.Square,
                                         accum_out=ssq[:, t:t + 1])
                st = work.tile([128, 4 * B], f32, tag="lnst", name=f"{pfx}st")
                mean, msq, var, nmr = (st[:, i * B:(i + 1) * B] for i in range(4))
                nc.vector.tensor_scalar_mul(mean, ssum[:], 1.0 / E)
                nc.vector.tensor_scalar_mul(msq, ssq[:], 1.0 / E)
                nc.vector.tensor_tensor(out=var, in0=mean, in1=mean, op=ALU.mult)
                nc.vector.tensor_tensor(out=var, in0=msq, in1=var, op=ALU.subtract)
                nc.scalar.activation(var, var, AF.Sqrt, bias=eps_sb[:], scale=1.0)
                nc.vector.reciprocal(out=var, in_=var)   # var now holds r
                nc.vector.tensor_tensor(out=nmr, in0=mean, in1=var, op=ALU.mult)
                nc.vector.tensor_scalar_mul(nmr, nmr, -1.0)
                xnT_v = xnT[:].rearrange("p (m s) -> p m s", m=8)
                for t in range(B):
                    xn = work.tile([128, E], f32, tag="lnscr", name="lnxn")
                    nc.vector.tensor_scalar(out=xn[:], in0=x[:, t * E:(t + 1) * E],
                                            scalar1=var[:, t:t + 1],
                                            scalar2=nmr[:, t:t + 1],
                                            op0=ALU.mult, op1=ALU.add)
                    for half in range(2):
                        tp = tp_pool.tile([128, 512], f32, tag="tp", name="tp")
                        for q in range(4):
                            m = half * 4 + q
                            nc.tensor.transpose(tp[:, q * 128:(q + 1) * 128],
                                                xn[:, m * 128:(m + 1) * 128],
                                                ident[:])
                        evac(xnT_v[:, half * 4:(half + 1) * 4,
                                   t * 128:(t + 1) * 128], tp[:])

            # ----- stage 1: load rows + LN1 -----
            x_rows = persist.tile([128, B * E], f32, tag="xrows", name="xrows")
            for t in range(B):
                dma(x_rows[:, t * E:(t + 1) * E], hid[t * 128:(t + 1) * 128, :])
            xnT = persist.tile([128, 8 * CH], bf16, tag="xnT", name="xnT")
            layer_norm_T(x_rows, xnT, "ln1")

            # ----- stage 2: AllGather normalized-transposed chunks (bf16) -----
            ag_in = dram.tile([E, CH], bf16)
            ag_out = dram.tile([W * E, CH], bf16, addr_space="Shared")
            for m in range(8):
                dma(ag_in[m * 128:(m + 1) * 128, :], xnT[:, m * CH:(m + 1) * CH])
            if mock_cc:
                nc.sync.dma_start(out=ag_out[0:E, :], in_=ag_in[:, :])
            else:
                nc.gpsimd.collective_compute(
                    "AllGather", ALU.bypass, replica_groups=rg,
                    ins=[ag_in.opt()], outs=[ag_out.opt()])

            # ----- stage 3: qkv for this core's 2 heads -----
            wqkv = persist.tile([128, 3 * 8 * 128], bf16, tag="wqkv", name="wqkv")
            dma(wqkv[:], qkv_w[:, :])

            qT = persist.tile([128, SS], dt_act, tag="qT", name="qT")
            kT = persist.tile([128, SS], dt_act, tag="kT", name="kT")
            vT = persist.tile([128, SS], dt_act, tag="vT", name="vT")
            qkvT = [qT, kT, vT]
            for cc in range(NQ):
                g0, g1 = cc * QC, (cc + 1) * QC
                xg = [xccp.tile([128, QC], bf16, tag=f"xcc{k}", name=f"xcc{k}")
                      for k in range(8)]
                for k in range(8):
                    j0, j1 = g0 // CH, (g1 - 1) // CH
                    for j in range(j0, j1 + 1):
                        a, b_ = max(g0, j * CH), min(g1, (j + 1) * CH)
                        dma(xg[k][:, a - g0:b_ - g0],
                            ag_out[j * E + k * 128:j * E + (k + 1) * 128,
                                   a - j * CH:b_ - j * CH])
                for c in range(3):
                    ps = acc_pool.tile([128, QC], f32, tag="acc", name="acc")
                    for k in range(8):
                        nc.tensor.matmul(
                            ps[:],
                            lhsT=wqkv[:, (c * 8 + k) * 128:(c * 8 + k + 1) * 128],
                            rhs=xg[k][:],
                            start=(k == 0),
                            stop=(k == 7 and not use_qkv_b))
                    if use_qkv_b:
                        nc.tensor.matmul(
                            ps[:], lhsT=qkv_b_sb[:, c * 128:(c + 1) * 128],
                            rhs=ones_row[:], start=False, stop=True)
                    evac(qkvT[c][:, g0:g1], ps[:])

            # ----- stage 4: V transposed + ones-augmented column -----
            v_aug = [persist.tile([128, NB * 65], dt_act, tag=f"vaug{h}",
                                  name=f"vaug{h}") for h in range(2)]
            for h in range(2):
                vv = v_aug[h][:].rearrange("p (n c) -> p n c", c=65)
                nc.vector.memset(vv[:, :, 64:65], 1.0)
            for g in range(NB // 4):
                tpv = tp_pool.tile([128, 512], dt_act, tag="tp", name="tpv")
                for q in range(4):
                    tb = g * 4 + q
                    nc.tensor.transpose(tpv[:, q * 128:(q + 1) * 128],
                                        vT[:, tb * 128:(tb + 1) * 128],
                                        ident_a[:])
                tps = tpv[:].rearrange("p (n c) -> p n c", c=128)
                for h in range(2):
                    vv = v_aug[h][:].rearrange("p (n c) -> p n c", c=65)
                    nc.vector.tensor_copy(
                        out=vv[:, g * 4:(g + 1) * 4, 0:64],
                        in_=tps[:, :, 64 * h:64 * h + 64])

            # ----- stage 5+6: attention, head-outer; per-head AllToAll -----
            a2a_in = [dram.tile([W * 64, CH], bf16, name=f"a2a_in{h}")
                      for h in range(2)]
            a2a_out = [dram.tile([W * 64, CH], bf16, name=f"a2a_out{h}")
                       for h in range(2)]
            for h in range(2):
                e0 = 64 * h
                for qc in range(NQ):
                    n_t = (QC // 128) * (qc + 1)
                    av = acc_pool.tile([65, QC], f32, tag="acc", name="avacc")
                    for tp2 in range(n_t // 2):
                        tb0 = tp2 * 2
                        sc = sc_pool.tile([128, 2 * QC], f32, tag="sc", name="sc")
                        for u in range(2):
                            tb = tb0 + u
                            nc.tensor.matmul(
                                sc[:, u * QC:(u + 1) * QC],
                                lhsT=kT[e0:e0 + 64, tb * 128:(tb + 1) * 128],
                                rhs=qT[e0:e0 + 64, qc * QC:(qc + 1) * QC],
                                start=True, stop=True)
                        ex = exp_pool.tile([128, 2 * QC], dt_act, tag="ex",
                                           name="ex")
                        if masked:
                            for u in range(2):
                                tb = tb0 + u
                                nc.scalar.activation(
                                    ex[:, u * QC:(u + 1) * QC],
                                    sc[:, u * QC:(u + 1) * QC], AF.Exp,
                                    bias=mb_sb[:, tb:tb + 1], scale=1.0)
                        else:
                            nc.scalar.activation(ex[:], sc[:], AF.Exp)
                        p0 = tb0 - (QC // 128) * qc
                        if p0 >= 0:
                            # zero future (t > s) lanes of the 2 diag blocks
                            nc.gpsimd.affine_select(
                                out=ex[:], in_=ex[:],
                                compare_op=ALU.is_ge,
                                fill=0.0,
                                base=-128 * p0,
                                channel_multiplier=-1,
                                pattern=[[-128, 2], [1, QC]],
                            )
                        for u in range(2):
                            tb = tb0 + u
                            nc.tensor.matmul(
                                av[:],
                                lhsT=v_aug[h][:, tb * 65:(tb + 1) * 65],
                                rhs=ex[:, u * QC:(u + 1) * QC],
                                start=(tb == 0), stop=(tb == n_t - 1))
                    recip = work.tile([1, QC], f32, tag="recip", name="recip")
                    nc.vector.reciprocal(out=recip[:], in_=av[64:65, :])
                    bc = work.tile([64, QC], f32, tag="bc", name="bc")
                    nc.gpsimd.partition_broadcast(bc[:], recip[:])
                    avs = work.tile([64, QC], bf16, tag="avsc", name="avsc")
                    nc.vector.tensor_tensor(out=avs[:], in0=av[0:64, :],
                                            in1=bc[:], op=ALU.mult)
                    g0, g1 = qc * QC, (qc + 1) * QC
                    for j in range(g0 // CH, (g1 - 1) // CH + 1):
                        a, b_ = max(g0, j * CH), min(g1, (j + 1) * CH)
                        dma(a2a_in[h][j * 64:(j + 1) * 64, a - j * CH:b_ - j * CH],
                            avs[:, a - g0:b_ - g0])
                if mock_cc:
                    nc.sync.dma_start(out=a2a_out[h][:, :], in_=a2a_in[h][:, :])
                else:
                    nc.gpsimd.collective_compute(
                        "AllToAll", ALU.bypass, replica_groups=rg,
                        ins=[a2a_in[h].opt()], outs=[a2a_out[h].opt()])

            # ----- stage 7: proj + residual -----
            wproj = persist.tile([128, 8 * 8 * 128], bf16, tag="wproj",
                                 name="wproj")
            dma(wproj[:], proj_w[:, :])
            aT = [persist.tile([128, CH], bf16, tag=f"aT{k}", name=f"aT{k}")
                  for k in range(8)]
            for k in range(8):
                dma(aT[k][0:64, :], a2a_out[0][k * 64:(k + 1) * 64, :])
                dma(aT[k][64:128, :], a2a_out[1][k * 64:(k + 1) * 64, :])
            res1 = persist.tile([128, B * E], f32, tag="res1", name="res1")
            res1_v = res1[:].rearrange("p (t e) -> p t e", e=E)
            xr_v = x_rows[:].rearrange("p (t e) -> p t e", e=E)
            for m in range(8):
                ps = acc_pool.tile([128, QC], f32, tag="acc", name="acc")
                for k in range(8):
                    nc.tensor.matmul(
                        ps[:, 0:CH],
                        lhsT=wproj[:, (m * 8 + k) * 128:(m * 8 + k + 1) * 128],
                        rhs=aT[k][:],
                        start=(k == 0), stop=(k == 7 and not use_proj_b))
                if use_proj_b:
                    nc.tensor.matmul(
                        ps[:, 0:CH], lhsT=proj_b_sb[:, m * 128:(m + 1) * 128],
                        rhs=ones_row[:, 0:CH], start=False, stop=True)
                pTm = work.tile([128, CH], f32, tag="pTm", name="pTm")
                evac(pTm[:], ps[:, 0:CH])
                tpp = tp_pool.tile([128, 512], f32, tag="tp", name="tpp")
                for t in range(B):
                    nc.tensor.transpose(tpp[:, t * 128:(t + 1) * 128],
                                        pTm[:, t * 128:(t + 1) * 128],
                                        ident[:])
                tps = tpp[:, 0:B * 128].rearrange("p (t e) -> p t e", e=128)
                nc.vector.tensor_tensor(
                    out=res1_v[:, :, m * 128:(m + 1) * 128],
                    in0=tps, in1=xr_v[:, :, m * 128:(m + 1) * 128],
                    op=ALU.add)

            # ----- stage 8: LN2 (ln2 affine folded into w1) -----
            l2T = persist.tile([128, 8 * CH], bf16, tag="l2T", name="l2T")
            layer_norm_T(res1, l2T, "ln2")

            # ----- stage 9: MLP (full, on this core's seq chunk; bf16) -----
            h1T = [persist.tile([128, CH], bf16, tag=f"h1T{m}", name=f"h1T{m}")
                   for m in range(32)]
            for m in range(32):
                w1m = wstream.tile([128, 8 * 128], bf16, tag="w1m", name="w1m")
                dma(w1m[:], w1[m * 128:(m + 1) * 128, :])
                ps = acc_pool.tile([128, QC], f32, tag="acc", name="acc")
                for k in range(8):
                    nc.tensor.matmul(
                        ps[:, 0:CH], lhsT=w1m[:, k * 128:(k + 1) * 128],
                        rhs=l2T[:, k * CH:(k + 1) * CH],
                        start=(k == 0), stop=(k == 7 and not use_mlp1_b))
                if use_mlp1_b:
                    nc.tensor.matmul(
                        ps[:, 0:CH], lhsT=b1_sb[:, m * 128:(m + 1) * 128],
                        rhs=ones_row[:, 0:CH], start=False, stop=True)
                if m % 2 == 0:
                    nc.scalar.activation(h1T[m][:], ps[:, 0:CH], AF.Relu)
                else:
                    nc.vector.tensor_scalar(out=h1T[m][:], in0=ps[:, 0:CH],
                                            scalar1=0.0, scalar2=None,
                                            op0=ALU.max)

            oT = [persist.tile([128, CH], bf16, tag=f"oT{m}", name=f"oT{m}")
                  for m in range(8)]
            for m in range(8):
                ps = acc_pool.tile([128, QC], f32, tag="acc", name="acc")
                for half in range(2):
                    w2m = w2stream.tile([128, 16 * 128], bf16, tag="w2m",
                                        name="w2m")
                    dma(w2m[:], w2[m * 128:(m + 1) * 128,
                                   half * 16 * 128:(half + 1) * 16 * 128])
                    for kk in range(16):
                        k = half * 16 + kk
                        nc.tensor.matmul(
                            ps[:, 0:CH], lhsT=w2m[:, kk * 128:(kk + 1) * 128],
                            rhs=h1T[k][:],
                            start=(k == 0), stop=(k == 31 and not use_mlp2_b))
                if use_mlp2_b:
                    nc.tensor.matmul(
                        ps[:, 0:CH], lhsT=b2_sb[:, m * 128:(m + 1) * 128],
                        rhs=ones_row[:, 0:CH], start=False, stop=True)
                evac(oT[m][:], ps[:, 0:CH])

            # ----- stage 10: transpose back + final residual + out -----
            for t in range(B):
                tpo = tp_pool.tile([128, 8 * 128], bf16, tag="tp", name="tpo")
                for m in range(8):
                    nc.tensor.transpose(tpo[:, m * 128:(m + 1) * 128],
                                        oT[m][:, t * 128:(t + 1) * 128],
                                        ident_bf[:])
                orow = work.tile([128, E], f32, tag="orow", name="orow")
                nc.vector.tensor_tensor(
                    out=orow[:], in0=tpo[:],
                    in1=res1[:, t * E:(t + 1) * E], op=ALU.add)
                dma(out[t * 128:(t + 1) * 128, :], orow[:])

    return nc


def _prepare_in_maps(inputs, SS: int):
    """Host-side prep: fold LN affines into the following matmuls (exact),
    slice per core, prescale q by 1/8, pre-tile all weights contiguously,
    bf16-cast matmul weights."""
    import ml_dtypes

    bf16 = ml_dtypes.bfloat16
    CH = SS // W
    NB = SS // 128
    hid = np.ascontiguousarray(
        np.asarray(inputs["hidden_states"], np.float32)[0, :SS])

    ln1_w = np.asarray(inputs["ln1_w"], np.float32)
    ln1_b = np.asarray(inputs["ln1_b"], np.float32)
    ln2_w = np.asarray(inputs["ln2_w"], np.float32)
    ln2_b = np.asarray(inputs["ln2_b"], np.float32)

    # fold LN1 into qkv conv
    attn_w = ln1_w[:, None] * np.asarray(inputs["attn_w"], np.float32)
    attn_b = (np.asarray(inputs["attn_b"], np.float32)
              + ln1_b @ np.asarray(inputs["attn_w"], np.float32))
    attn_w[:, :E] *= 0.125
    attn_b[:E] *= 0.125

    # fold LN2 into mlp w1
    w1 = ln2_w[:, None] * np.asarray(inputs["mlp_w1"], np.float32)
    b1 = (np.asarray(inputs["mlp_b1"], np.float32)
          + ln2_b @ np.asarray(inputs["mlp_w1"], np.float32))

    proj_w = np.asarray(inputs["proj_w"], np.float32)
    proj_b = np.asarray(inputs["proj_b"], np.float32)
    w2 = np.asarray(inputs["mlp_w2"], np.float32)
    b2 = np.asarray(inputs["mlp_b2"], np.float32)

    mask = np.asarray(inputs["mask"])[0, 0, 0, :SS]
    masked = not bool(mask.all())
    mask_bias = np.where(mask, 0.0, MASK).astype(np.float32)

    # X[k*128+p, m*128+f] -> [(m p), (k f)]  (k-major inside a row-block)
    def tile_mk(x, km, mm_):
        return np.ascontiguousarray(
            x.reshape(km, 128, mm_, 128).transpose(2, 1, 0, 3)
            .reshape(mm_ * 128, km * 128))

    # proj tiled as [128, m, k, 128] single row-block for one contiguous DMA
    proj_t = (proj_w.reshape(8, 128, 8, 128).transpose(1, 2, 0, 3)
              .reshape(128, 8 * 8 * 128))

    biases = (bool(np.any(attn_b)), bool(np.any(proj_b)),
              bool(np.any(b1)), bool(np.any(b2)))

    common = {
        "proj_w": np.ascontiguousarray(proj_t).astype(bf16),
        "proj_b": np.ascontiguousarray(proj_b.reshape(1, -1)),
        "w1": tile_mk(w1, 8, 32).astype(bf16),
        "b1": np.ascontiguousarray(b1.reshape(1, -1)),
        "w2": tile_mk(w2, 32, 8).astype(bf16),
        "b2": np.ascontiguousarray(b2.reshape(1, -1)),
        "mask_bias": np.ascontiguousarray(mask_bias.reshape(NB, 128).T),
    }
    in_maps = []
    for i in range(W):
        wq = np.empty((128, 3, 8, 128), np.float32)
        bq = np.empty((3, 128), np.float32)
        for c in range(3):
            cols = slice(c * E + 128 * i, c * E + 128 * i + 128)
            wq[:, c] = attn_w[:, cols].reshape(8, 128, 128).transpose(1, 0, 2)
            bq[c] = attn_b[cols]
        in_maps.append({
            "hidden": np.ascontiguousarray(hid[i * CH:(i + 1) * CH]),
            "qkv_w": np.ascontiguousarray(wq.reshape(128, -1)).astype(bf16),
            "qkv_b": np.ascontiguousarray(bq.reshape(1, -1)),
            **common,
        })
    return in_maps, masked, biases


def _run(inputs, SS, dt_name, **kw):
    from concourse.bass_utils import run_bass_kernel_spmd

    in_maps, masked, biases = _prepare_in_maps(inputs, SS)
    key = (SS, dt_name, masked, biases)
    if key not in _CACHE:
        nc = _build(SS, dt_name, masked=masked, biases=biases)
        nc.finalize()
        _CACHE[key] = nc
    nc = _CACHE[key]
    res = run_bass_kernel_spmd(nc, in_maps, core_ids=list(range(W)), **kw)
    full = np.concatenate([r["out"] for r in res.results], axis=0)
    return full[None].astype(np.float32), res


def kernel(**inputs) -> np.ndarray:
    out, _ = _run(inputs, 3072, "bf16")
    return out
